# revision 5
# baseline (speedup 1.0000x reference)
"""Trainium2 Bass kernel for nn_EquivariantToyResFlow.

Math: 8 i-ResBlocks g(x) = relu-MLP(2->256->256->256->256->2), residual
z <- z + g(z); logdet via 5-term Hutchinson power series sum_k c_k v^T J^k v.
Because g is piecewise-linear, J is an exact per-sample 2x2 matrix obtained
from 2 "basis" JVP chains (e0, e1); the power series then collapses to
per-sample 2x2 algebra done once per block over the whole shard.

Layout: features on partitions, batch on the free dim ([256, n] tiles as
[128, 2*n]).  Forward chain runs in fp32 (exact -> relu masks match the
fp32 reference; masks are discrete so TF32-class rounding there is
catastrophic).  Basis chains run in float32r (TF32-ish, 4x faster) since
their error is smooth.  Per-sample 2x2 power series runs transposed
(batch on partitions) so the DVE uses all 128 lanes.

Sharding: pure data parallel, batch 131072 -> 8 cores x 16384.
"""
import numpy as np

import concourse.bass as bass
import concourse.mybir as mybir
import concourse.tile as tile
from concourse import bacc
from concourse.bass_utils import run_bass_kernel_spmd
from concourse.masks import make_identity

F32 = mybir.dt.float32
F32R = mybir.dt.float32r
BF16 = mybir.dt.bfloat16
AF = mybir.ActivationFunctionType
OP = mybir.AluOpType

B = 131072
NCORES = 8
S = B // NCORES          # 16384 samples per core
NB = 8                   # res blocks
NMID = 3
NPS = 5
MACRO = 512              # samples per loop iteration
NITER = S // MACRO       # 16
NSUB = MACRO // 512      # matmul column sub-tiles
NCH = MACRO // 128       # 8 sample chunks of 128
NCHG = S // 128          # 128 global chunks

_CACHE = {}


def _emit(nc, tc, t_x, t_v, t_Wi, t_bi, t_Wm, t_bm, t_Wo, t_bo, t_z, t_dlp):
    x_d, v_d = t_x.ap(), t_v.ap()
    Wi_d, bi_d, Wm_d, bm_d = t_Wi.ap(), t_bi.ap(), t_Wm.ap(), t_bm.ap()
    Wo_d, bo_d = t_Wo.ap(), t_bo.ap()
    z_d, dlp_d = t_z.ap(), t_dlp.ap()

    with tc.tile_pool(name="const", bufs=1) as cpool, \
         tc.tile_pool(name="hpool", bufs=2) as hpool, \
         tc.tile_pool(name="tpool", bufs=3) as tpool, \
         tc.tile_pool(name="mpool", bufs=4) as mpool, \
         tc.tile_pool(name="small", bufs=4) as small, \
         tc.tile_pool(name="pw", bufs=1) as pw, \
         tc.tile_pool(name="upsum", bufs=4, space="PSUM") as upsum, \
         tc.tile_pool(name="spsum", bufs=2, space="PSUM") as spsum:

        ident = cpool.tile([2, 2], F32)
        make_identity(nc, ident[:])

        # ---------------- weights / biases (SBUF resident) ----------------
        Wm32 = cpool.tile([128, NB * NMID * 4 * 128], F32)
        Wmr = cpool.tile([128, NB * NMID * 4 * 128], F32R)
        for b in range(NB):
            src = Wm_d[b].rearrange("m (kc kp) (mc mm) -> kp m kc mc mm",
                                    kp=128, mm=128)
            for dst, eng in ((Wm32, nc.sync), (Wmr, nc.gpsimd)):
                eng.dma_start(
                    out=dst[:, b * NMID * 512:(b + 1) * NMID * 512].rearrange(
                        "p (m kc mc mm) -> p m kc mc mm", m=NMID, kc=2, mc=2, mm=128),
                    in_=src)
        Wi_sb = cpool.tile([2, NB * 2 * 128], F32)      # [k, (b mc mm)]
        nc.sync.dma_start(
            out=Wi_sb[:].rearrange("k (b mc mm) -> k b mc mm", b=NB, mc=2, mm=128),
            in_=Wi_d.rearrange("b k (mc mm) -> k b mc mm", mm=128))
        WiT_sb = cpool.tile([128, NB * 2 * 2], F32)     # [fp, (b k fc)]
        nc.sync.dma_start(
            out=WiT_sb[:].rearrange("p (b k fc) -> p b k fc", b=NB, k=2, fc=2),
            in_=Wi_d.rearrange("b k (fc fp) -> fp b k fc", fp=128))
        Wo_sb = cpool.tile([128, NB * 2 * 2], F32)      # [kp, (b kc f)]
        nc.sync.dma_start(
            out=Wo_sb[:].rearrange("p (b kc f) -> p b kc f", b=NB, kc=2, f=2),
            in_=Wo_d.rearrange("b (kc kp) f -> kp b kc f", kp=128))
        Wor_sb = cpool.tile([128, NB * 2 * 2], F32R)
        nc.gpsimd.dma_start(
            out=Wor_sb[:].rearrange("p (b kc f) -> p b kc f", b=NB, kc=2, f=2),
            in_=Wo_d.rearrange("b (kc kp) f -> kp b kc f", kp=128))
        bi_sb = cpool.tile([128, NB * 2], F32)          # [p, (b c)]
        nc.sync.dma_start(
            out=bi_sb[:].rearrange("p (b c) -> p b c", b=NB, c=2),
            in_=bi_d.rearrange("b (c p) -> p b c", p=128))
        bm_sb = cpool.tile([128, NB * NMID * 2], F32)   # [p, (b m c)]
        nc.sync.dma_start(
            out=bm_sb[:].rearrange("p (b m c) -> p b m c", b=NB, m=NMID, c=2),
            in_=bm_d.rearrange("b m (c p) -> p b m c", p=128))
        bo_sb = cpool.tile([2, NB], F32)                # [f, b]
        nc.sync.dma_start(out=bo_sb[:], in_=bo_d.rearrange("b f -> f b"))

        # ---------------- I/O state ----------------
        xv = x_d.rearrange("n f -> f n")    # [2, S] strided dram views
        zv = z_d.rearrange("n f -> f n")
        vT = cpool.tile([128, NCHG * 2], F32)           # [p, (c f)]
        nc.sync.dma_start(
            out=vT[:].rearrange("p (c f) -> p c f", c=NCHG, f=2),
            in_=v_d.rearrange("(c p) f -> p c f", p=128))
        # J storage: per block [p, (chg e)] with e = (J00, J01, J10, J11)
        Jst = [cpool.tile([128, NCHG * 4], F32, name=f"Jst{b}", tag=f"J{b}")
               for b in range(NB)]

        def wm_sl(dst, b, m, kc, mc):
            base = (((b * NMID + m) * 2 + kc) * 2 + mc) * 128
            return dst[:, base:base + 128]

        # ---------------- main loop over macro-tiles ----------------
        with tc.For_i(0, NITER, 1) as it:
            z_sl = small.tile([2, MACRO], F32, tag="z")
            nc.sync.dma_start(out=z_sl[:], in_=xv[:, bass.ds(it * MACRO, MACRO)])

            for b in range(NB):
                # ===== forward chain (fp32, exact masks) =====
                h1 = hpool.tile([128, 2 * MACRO], F32, tag="h")
                for mc in range(2):
                    pt = upsum.tile([128, MACRO], F32, tag="u")
                    for s in range(NSUB):
                        nc.tensor.matmul(
                            pt[:, s * 512:(s + 1) * 512],
                            Wi_sb[:, (b * 2 + mc) * 128:(b * 2 + mc + 1) * 128],
                            z_sl[:, s * 512:(s + 1) * 512],
                            start=True, stop=True)
                    nc.scalar.activation(h1[:, mc * MACRO:(mc + 1) * MACRO], pt[:],
                                         AF.Relu, bias=bi_sb[:, b * 2 + mc:b * 2 + mc + 1])
                m1 = mpool.tile([128, 2 * MACRO], BF16, tag="mask")
                for mc in range(2):
                    nc.gpsimd.tensor_scalar(m1[:, mc * MACRO:(mc + 1) * MACRO],
                                            h1[:, mc * MACRO:(mc + 1) * MACRO],
                                            0.0, None, OP.is_gt)
                hprev = h1
                masks = [m1]
                for j in range(NMID):
                    hn = hpool.tile([128, 2 * MACRO], F32, tag="h")
                    mt = mpool.tile([128, 2 * MACRO], BF16, tag="mask")
                    for mc in range(2):
                        pt = upsum.tile([128, MACRO], F32, tag="u")
                        for kc in range(2):
                            for s in range(NSUB):
                                nc.tensor.matmul(
                                    pt[:, s * 512:(s + 1) * 512],
                                    wm_sl(Wm32, b, j, kc, mc),
                                    hprev[:, kc * MACRO + s * 512:
                                          kc * MACRO + (s + 1) * 512],
                                    start=(kc == 0), stop=(kc == 1))
                        nc.scalar.activation(
                            hn[:, mc * MACRO:(mc + 1) * MACRO], pt[:], AF.Relu,
                            bias=bm_sb[:, (b * NMID + j) * 2 + mc:
                                       (b * NMID + j) * 2 + mc + 1])
                        nc.gpsimd.tensor_scalar(mt[:, mc * MACRO:(mc + 1) * MACRO],
                                                hn[:, mc * MACRO:(mc + 1) * MACRO],
                                                0.0, None, OP.is_gt)
                    masks.append(mt)
                    hprev = hn
                h3 = hprev

                # forward out layer (fp32): gx = relu(h3 @ Wo + bo)
                po = spsum.tile([2, MACRO], F32, tag="row")
                for kc in range(2):
                    for s in range(NSUB):
                        nc.tensor.matmul(
                            po[:, s * 512:(s + 1) * 512],
                            Wo_sb[:, (b * 2 + kc) * 2:(b * 2 + kc) * 2 + 2],
                            h3[:, kc * MACRO + s * 512: kc * MACRO + (s + 1) * 512],
                            start=(kc == 0), stop=(kc == 1))
                gx = small.tile([2, MACRO], F32, tag="gx")
                nc.scalar.activation(gx[:], po[:], AF.Relu,
                                     bias=bo_sb[:, b:b + 1])
                mo = small.tile([2, MACRO], F32, tag="mo")
                nc.scalar.sign(mo[:], gx[:])
                # z <- z + gx
                nc.vector.tensor_tensor(z_sl[:], z_sl[:], gx[:], OP.add)

                # ===== basis chains (f32r) =====
                tprev = []
                for c in range(2):
                    t1 = tpool.tile([128, 2 * MACRO], F32R, tag="t")
                    for fc in range(2):
                        nc.vector.tensor_scalar(
                            t1[:, fc * MACRO:(fc + 1) * MACRO],
                            masks[0][:, fc * MACRO:(fc + 1) * MACRO],
                            WiT_sb[:, (b * 2 + c) * 2 + fc:(b * 2 + c) * 2 + fc + 1],
                            None, OP.mult)
                    tprev.append(t1)
                for j in range(NMID):
                    for c in range(2):
                        tn = tpool.tile([128, 2 * MACRO], F32R, tag="t")
                        for mc in range(2):
                            pt = upsum.tile([128, MACRO], F32, tag="u")
                            for kc in range(2):
                                for s in range(NSUB):
                                    nc.tensor.matmul(
                                        pt[:, s * 512:(s + 1) * 512],
                                        wm_sl(Wmr, b, j, kc, mc),
                                        tprev[c][:, kc * MACRO + s * 512:
                                                 kc * MACRO + (s + 1) * 512],
                                        start=(kc == 0), stop=(kc == 1))
                            nc.vector.tensor_tensor(
                                tn[:, mc * MACRO:(mc + 1) * MACRO], pt[:],
                                masks[j + 1][:, mc * MACRO:(mc + 1) * MACRO],
                                OP.mult)
                        tprev[c] = tn

                # ===== J rows + masking + transpose into Jst =====
                Jrows = small.tile([2, 2 * MACRO], F32, tag="Jrows")
                for c in range(2):
                    pj = spsum.tile([2, MACRO], F32, tag="row")
                    for kc in range(2):
                        for s in range(NSUB):
                            nc.tensor.matmul(
                                pj[:, s * 512:(s + 1) * 512],
                                Wor_sb[:, (b * 2 + kc) * 2:(b * 2 + kc) * 2 + 2],
                                tprev[c][:, kc * MACRO + s * 512:
                                         kc * MACRO + (s + 1) * 512],
                                start=(kc == 0), stop=(kc == 1))
                    nc.vector.tensor_tensor(Jrows[:, c * MACRO:(c + 1) * MACRO],
                                            pj[:], mo[:], OP.mult)
                for ch in range(NCH):
                    for c in range(2):
                        pjt = spsum.tile([128, 2], F32, tag="pjt")
                        nc.tensor.transpose(
                            pjt[:],
                            Jrows[:, c * MACRO + ch * 128: c * MACRO + (ch + 1) * 128],
                            ident[:])
                        nc.vector.tensor_copy(
                            Jst[b][:, bass.ds(it * (NCH * 4) + ch * 4 + c * 2, 2)],
                            pjt[:])

            nc.sync.dma_start(out=zv[:, bass.ds(it * MACRO, MACRO)], in_=z_sl[:])

        # ---------------- power series (batch on partitions) ----------------
        def jent(b, e):
            return Jst[b][:].rearrange("p (c e) -> p e c", e=4)[:, e, :]

        def vent(f):
            return vT[:].rearrange("p (c f) -> p f c", f=2)[:, f, :]

        w0 = pw.tile([128, NCHG], F32)
        w1 = pw.tile([128, NCHG], F32)
        acc = pw.tile([128, NCHG], F32)
        tA = pw.tile([128, NCHG], F32)
        tB = pw.tile([128, NCHG], F32)
        nc.gpsimd.memset(acc[:], 0.0)
        for b in range(NB):
            nc.vector.tensor_copy(w0[:], vent(0))
            nc.vector.tensor_copy(w1[:], vent(1))
            for k in range(1, NPS + 1):
                coef = -((-1.0) ** (k + 1)) / k    # minus for delta_logp
                nc.vector.tensor_tensor(tA[:], w0[:], jent(b, 0), OP.mult)
                nc.vector.tensor_tensor(tB[:], w1[:], jent(b, 2), OP.mult)
                nc.vector.tensor_tensor(tA[:], tA[:], tB[:], OP.add)   # new w0
                nc.vector.tensor_tensor(tB[:], w0[:], jent(b, 1), OP.mult)
                nc.vector.tensor_tensor(w1[:], w1[:], jent(b, 3), OP.mult)
                nc.vector.tensor_tensor(w1[:], tB[:], w1[:], OP.add)   # new w1
                nc.vector.tensor_copy(w0[:], tA[:])
                nc.vector.tensor_tensor(tA[:], w0[:], vent(0), OP.mult)
                nc.vector.tensor_tensor(tB[:], w1[:], vent(1), OP.mult)
                nc.vector.tensor_tensor(tA[:], tA[:], tB[:], OP.add)
                nc.vector.tensor_scalar(tA[:], tA[:], coef, None, OP.mult)
                nc.vector.tensor_tensor(acc[:], acc[:], tA[:], OP.add)

        # ---------------- outputs ----------------
        nc.sync.dma_start(out=dlp_d.rearrange("(c p) one -> p (c one)", p=128),
                          in_=acc[:])


def _build():
    nc = bacc.Bacc("TRN2", target_bir_lowering=False, debug=False)
    t_x = nc.dram_tensor("x_in", [S, 2], F32, kind="ExternalInput")
    t_v = nc.dram_tensor("v_in", [S, 2], F32, kind="ExternalInput")
    t_Wi = nc.dram_tensor("Wi_in", [NB, 2, 256], F32, kind="ExternalInput")
    t_bi = nc.dram_tensor("bi_in", [NB, 256], F32, kind="ExternalInput")
    t_Wm = nc.dram_tensor("Wm_in", [NB, NMID, 256, 256], F32, kind="ExternalInput")
    t_bm = nc.dram_tensor("bm_in", [NB, NMID, 256], F32, kind="ExternalInput")
    t_Wo = nc.dram_tensor("Wo_in", [NB, 256, 2], F32, kind="ExternalInput")
    t_bo = nc.dram_tensor("bo_in", [NB, 2], F32, kind="ExternalInput")
    t_z = nc.dram_tensor("z_out", [S, 2], F32, kind="ExternalOutput")
    t_dlp = nc.dram_tensor("dlp_out", [S, 1], F32, kind="ExternalOutput")

    with tile.TileContext(nc) as tc:
        _emit(nc, tc, t_x, t_v, t_Wi, t_bi, t_Wm, t_bm, t_Wo, t_bo, t_z, t_dlp)
    nc.compile()
    return nc


def kernel(x, v, Wi, bi, Wm, bm, Wo, bo, _trace=False):
    x = np.ascontiguousarray(np.asarray(x, dtype=np.float32))
    v = np.ascontiguousarray(np.asarray(v, dtype=np.float32))
    Wi = np.ascontiguousarray(np.asarray(Wi, dtype=np.float32))
    bi = np.ascontiguousarray(np.asarray(bi, dtype=np.float32))
    Wm = np.ascontiguousarray(np.asarray(Wm, dtype=np.float32))
    bm = np.ascontiguousarray(np.asarray(bm, dtype=np.float32))
    Wo = np.ascontiguousarray(np.asarray(Wo, dtype=np.float32))
    bo = np.ascontiguousarray(np.asarray(bo, dtype=np.float32))

    if "nc" not in _CACHE:
        _CACHE["nc"] = _build()
    nc = _CACHE["nc"]

    in_maps = []
    for c in range(NCORES):
        sl = slice(c * S, (c + 1) * S)
        in_maps.append({
            "x_in": x[sl], "v_in": v[sl],
            "Wi_in": Wi, "bi_in": bi, "Wm_in": Wm, "bm_in": bm,
            "Wo_in": Wo, "bo_in": bo,
        })
    try:
        res = run_bass_kernel_spmd(nc, in_maps, core_ids=list(range(NCORES)),
                                   trace=_trace)
    except ModuleNotFoundError:
        res = run_bass_kernel_spmd(nc, in_maps, core_ids=list(range(NCORES)),
                                   trace=False)
    z = np.concatenate([res.results[c]["z_out"] for c in range(NCORES)], axis=0)
    dlp = np.concatenate([res.results[c]["dlp_out"] for c in range(NCORES)], axis=0)
    if _trace:
        _CACHE["last_exec_time_ns"] = res.exec_time_ns
        _CACHE["last_results"] = res
    return z, dlp
